# revision 37
# baseline (speedup 1.0000x reference)
"""Trainium2 Bass kernel for BilinearDecoder (v5).

score = sigmoid( einsum('ed,ed->e', z[edges[0]] @ W, z[edges[1]]) )

The kernel is bound by SWDGE descriptor generation on GPSIMD (~8ns per
gather descriptor, engine-serial; element SIZE is free).  v5 cuts row
descriptors ~2x by fetching edge PAIRS with one 2KB descriptor:

  Host sorts each core's edges by row; adjacent sorted edges have row
  gap 0 or 1 ~95% of the time.  A doubled table zw2 (flat 1KB rows:
  zw2[2i]=zW[i], zw2[2i+1]=zW[i]) serves both pair types with one
  overlapped-stride gather (elem 2KB, stride 1KB):
     idx 2r   -> [zW_r, zW_r ]   (equal-row pair)
     idx 2r+1 -> [zW_r, zW_r+1]  (consecutive-row pair)
  Pairs land as two 1KB halves in one partition; the col gather's index
  list is slot-permuted so each edge's z[col] row lands at the matching
  slot.  15 chunks x 512 pair-descs + 1 chunk x 1024 single-descs
  (P_FIX=7680 pairs, statically shaped; host falls back to the plain
  variant if an input pairs poorly) + 16 x 1024 col descs
  = 25088 descriptors vs 32768 plain.

  Phase 1 computes zW = z @ W in fp16 and writes each 128-node block
  twice (even/odd strided) into zw2; row-gather chunk k only reads a
  zw2 prefix (host-verified static bounds), so row gathers overlap the
  matmul via Tile's range-granular DRAM deps.  Per-edge dot: DVE f16
  mul + DVE tensor_reduce (keeps ACT off the critical path), ACT
  sigmoid at the end.
"""

import sys

if "/opt/trn_rl_repo" not in sys.path:
    sys.path.insert(0, "/opt/trn_rl_repo")

import numpy as np

N_NODES = 10000
N_NODES_PAD = 10240  # pad to multiple of 128
W_DIM = 512
N_EDGES = 131072
N_CORES = 8
EC = N_EDGES // N_CORES  # 16384 edges per core
CHUNK = 1024  # edge slots per chunk
NCHUNK = EC // CHUNK  # 16
NBLK = EC // 128  # 128 score columns per core
P_FIX = 7680  # static pair count: 15 chunks x 512 pairs
NPC = P_FIX // 512  # 15 pair chunks
S_FIX = EC - 2 * P_FIX  # 1024 single edges, 1 chunk
SB0_NODES = 7680  # zw2 prefix bound for the first 512 singles

_cache = {}


def _chunk_bounds():
    """Static per-pair-chunk zw prefix bounds (in nodes, mult of 128).

    Pair chunk k covers sorted edges up to ~(k+1)*CHUNK*(EC/(2*P_FIX)),
    i.e. rows below roughly that quantile.  +768 margin is >10 sigma of
    the order-statistic fluctuation; the host verifies per input and
    falls back to the plain variant."""
    bs = []
    for k in range(NPC):
        frac = (k + 1) * 2 * 512 / EC * (EC / (2 * P_FIX))  # = (k+1)/15
        b = int(np.ceil((N_NODES_PAD * frac + 768) / 128.0) * 128)
        bs.append(min(N_NODES_PAD, b))
    return bs


def _build_paired():
    import concourse.bacc as bacc
    import concourse.tile as tile
    from concourse import mybir
    import bass_rust

    f32 = mybir.dt.float32
    f16 = mybir.dt.float16
    i16 = mybir.dt.int16

    nc = bacc.Bacc(
        "TRN2",
        target_bir_lowering=False,
        debug=False,
        num_devices=N_CORES,
        dynamic_dma_scratch_size=16384,
    )
    zt = nc.dram_tensor("zt", [W_DIM, N_NODES_PAD], f16, kind="ExternalInput")
    ztbl = nc.dram_tensor("ztbl", [N_NODES_PAD, W_DIM], f16, kind="ExternalInput")
    w = nc.dram_tensor("w", [W_DIM, W_DIM], f16, kind="ExternalInput")
    # ridx: NPC*512 pair idxs then S_FIX single idxs (into zw2 flat rows)
    ridx = nc.dram_tensor(
        "ridx", [128, (P_FIX + S_FIX) // 16], i16, kind="ExternalInput"
    )
    cidx = nc.dram_tensor("cidx", [128, EC // 16], i16, kind="ExternalInput")
    zw2 = nc.dram_tensor("zw2", [2 * N_NODES_PAD, W_DIM], f16, kind="Internal")
    out = nc.dram_tensor("scores", [128, NBLK], f32, kind="ExternalOutput")

    def zw2_pair_view(bound_nodes):
        """Overlapped view: rows of 1024 f16 at 512-elem stride -> 2KB
        fetch at 1KB granularity.  Last row must not run off the buffer,
        so the view holds 2*bound-1 rows (idx <= 2*bound-2)."""
        nrows = 2 * bound_nodes - 1
        ap = zw2[:].copy()
        ap.ap = bass_rust.VecI64Pair([[W_DIM, nrows], [1, 2 * W_DIM]])
        return ap

    with tile.TileContext(nc) as tc:
        with (
            tc.tile_pool(name="wpool", bufs=1) as wpool,
            tc.tile_pool(name="zpanel", bufs=2) as zpool,
            tc.tile_pool(name="zwstage", bufs=6) as zwpool,
            tc.tile_pool(name="idx", bufs=1) as idxpool,
            tc.tile_pool(name="rgath", bufs=4) as rpool,
            tc.tile_pool(name="cgath", bufs=6) as cpool,
            tc.tile_pool(name="prod", bufs=4) as prodpool,
            tc.tile_pool(name="misc", bufs=1) as mpool,
            tc.tile_pool(name="psum1", bufs=6, space="PSUM") as psum1,
        ):
            # idx tables first so the first col gather can start ASAP
            ridx_sb = idxpool.tile([128, (P_FIX + S_FIX) // 16], i16, tag="ridx")
            nc.sync.dma_start(ridx_sb[:], ridx[:])
            cidx_sb = idxpool.tile([128, EC // 16], i16, tag="cidx")
            nc.sync.dma_start(cidx_sb[:], cidx[:])

            w_tiles = []
            for k in range(4):
                wt = wpool.tile([128, W_DIM], f16, tag=f"w{k}")
                nc.sync.dma_start(wt[:], w[k * 128 : (k + 1) * 128, :])
                w_tiles.append(wt)

            scores = mpool.tile([128, NBLK], f32, tag="scores")

            # ---- Phase 1: zW = z @ W (fp16), doubled into zw2 ----
            PANEL = 512
            for p in range(N_NODES_PAD // PANEL):
                zp = []
                for k in range(4):
                    t = zpool.tile([128, PANEL], f16, tag=f"zp{k}")
                    nc.sync.dma_start(
                        t[:], zt[k * 128 : (k + 1) * 128, p * PANEL : (p + 1) * PANEL]
                    )
                    zp.append(t)
                for ntile in range(PANEL // 128):
                    ps = psum1.tile([128, W_DIM], f32, tag="ps")
                    for k in range(4):
                        nc.tensor.matmul(
                            ps[:],
                            lhsT=zp[k][:, ntile * 128 : (ntile + 1) * 128],
                            rhs=w_tiles[k][:],
                            start=(k == 0),
                            stop=(k == 3),
                        )
                    st2 = zwpool.tile([128, 2 * W_DIM], f16, tag="zwst")
                    # ACT casts psum->f16 twice into [zw_i | zw_i] halves;
                    # partition p's 2KB then lands as zw2 rows 2i, 2i+1 in
                    # one contiguous DMA (keeps the Sync queue short).
                    nc.scalar.activation(
                        st2[:, 0:W_DIM], ps[:], mybir.ActivationFunctionType.Copy
                    )
                    nc.scalar.activation(
                        st2[:, W_DIM:], ps[:], mybir.ActivationFunctionType.Copy
                    )
                    node0 = p * PANEL + ntile * 128
                    nc.sync.dma_start(zw2[2 * node0 : 2 * node0 + 256, :], st2[:])

            # ---- Phase 2: paired/single row gathers + col gathers ----
            bounds = _chunk_bounds()
            for ch in range(NCHUNK):
                ct = cpool.tile([128, CHUNK // 128, W_DIM], f16, tag="ct")
                nc.gpsimd.dma_gather(
                    ct[:],
                    ztbl[:],
                    cidx_sb[:, ch * (CHUNK // 16) : (ch + 1) * (CHUNK // 16)],
                    CHUNK,
                    CHUNK,
                    W_DIM,
                )
                scr = prodpool.tile([128, CHUNK // 128, W_DIM], f16, tag="scr")
                if ch < NPC:
                    rt = rpool.tile([128, 4, 2 * W_DIM], f16, tag="rtp")
                    nc.gpsimd.dma_gather(
                        rt[:],
                        zw2_pair_view(bounds[ch]),
                        ridx_sb[:, ch * 32 : (ch + 1) * 32],
                        512,
                        512,
                        2 * W_DIM,
                        elem_step=W_DIM,
                    )
                    nc.vector.tensor_mul(
                        scr[:, 0:8:2, :], rt[:, :, 0:W_DIM], ct[:, 0:8:2, :]
                    )
                    nc.vector.tensor_mul(
                        scr[:, 1:8:2, :], rt[:, :, W_DIM:], ct[:, 1:8:2, :]
                    )
                else:
                    # two 512-desc single gathers; the first only needs a
                    # zw2 prefix (singles are row-sorted), so just the
                    # second waits for the full table.
                    rt1 = rpool.tile([128, 4, W_DIM], f16, tag="rts0")
                    nc.gpsimd.dma_gather(
                        rt1[:],
                        zw2[: 2 * SB0_NODES, :],
                        ridx_sb[:, NPC * 32 : NPC * 32 + 32],
                        512,
                        512,
                        W_DIM,
                    )
                    rt2 = rpool.tile([128, 4, W_DIM], f16, tag="rts1")
                    nc.gpsimd.dma_gather(
                        rt2[:],
                        zw2[:],
                        ridx_sb[:, NPC * 32 + 32 : NPC * 32 + 64],
                        512,
                        512,
                        W_DIM,
                    )
                    nc.vector.tensor_mul(scr[:, 0:4, :], rt1[:], ct[:, 0:4, :])
                    nc.vector.tensor_mul(scr[:, 4:8, :], rt2[:], ct[:, 4:8, :])
                nc.vector.tensor_reduce(
                    scores[:, ch * 8 : (ch + 1) * 8],
                    scr[:],
                    mybir.AxisListType.X,
                    mybir.AluOpType.add,
                )

            sig = mpool.tile([128, NBLK], f32, tag="sig")
            nc.scalar.activation(
                sig[:], scores[:], mybir.ActivationFunctionType.Sigmoid
            )
            nc.sync.dma_start(out[:], sig[:])

    nc.compile()
    return nc


def _build_plain():
    """Fallback for inputs that pair poorly: 32x1024 plain gathers
    (v1-style data path, fp16 phase 1, full-range row gathers)."""
    import concourse.bacc as bacc
    import concourse.tile as tile
    from concourse import mybir

    f32 = mybir.dt.float32
    f16 = mybir.dt.float16
    i16 = mybir.dt.int16

    nc = bacc.Bacc(
        "TRN2",
        target_bir_lowering=False,
        debug=False,
        num_devices=N_CORES,
        dynamic_dma_scratch_size=16384,
    )
    zt = nc.dram_tensor("zt", [W_DIM, N_NODES_PAD], f16, kind="ExternalInput")
    ztbl = nc.dram_tensor("ztbl", [N_NODES_PAD, W_DIM], f16, kind="ExternalInput")
    w = nc.dram_tensor("w", [W_DIM, W_DIM], f16, kind="ExternalInput")
    ridx = nc.dram_tensor("ridx", [128, EC // 16], i16, kind="ExternalInput")
    cidx = nc.dram_tensor("cidx", [128, EC // 16], i16, kind="ExternalInput")
    zw = nc.dram_tensor("zw", [N_NODES_PAD, W_DIM], f16, kind="Internal")
    out = nc.dram_tensor("scores", [128, NBLK], f32, kind="ExternalOutput")

    with tile.TileContext(nc) as tc:
        with (
            tc.tile_pool(name="wpool", bufs=1) as wpool,
            tc.tile_pool(name="zpanel", bufs=2) as zpool,
            tc.tile_pool(name="zwstage", bufs=4) as zwpool,
            tc.tile_pool(name="idx", bufs=1) as idxpool,
            tc.tile_pool(name="rgath", bufs=2) as rpool,
            tc.tile_pool(name="cgath", bufs=2) as cpool,
            tc.tile_pool(name="prod", bufs=2) as prodpool,
            tc.tile_pool(name="misc", bufs=1) as mpool,
            tc.tile_pool(name="psum1", bufs=4, space="PSUM") as psum1,
        ):
            w_tiles = []
            for k in range(4):
                wt = wpool.tile([128, W_DIM], f16, tag=f"w{k}")
                nc.sync.dma_start(wt[:], w[k * 128 : (k + 1) * 128, :])
                w_tiles.append(wt)
            ridx_sb = idxpool.tile([128, EC // 16], i16, tag="ridx")
            nc.sync.dma_start(ridx_sb[:], ridx[:])
            cidx_sb = idxpool.tile([128, EC // 16], i16, tag="cidx")
            nc.sync.dma_start(cidx_sb[:], cidx[:])
            scores = mpool.tile([128, NBLK], f32, tag="scores")
            PANEL = 512
            for p in range(N_NODES_PAD // PANEL):
                zp = []
                for k in range(4):
                    t = zpool.tile([128, PANEL], f16, tag=f"zp{k}")
                    nc.sync.dma_start(
                        t[:], zt[k * 128 : (k + 1) * 128, p * PANEL : (p + 1) * PANEL]
                    )
                    zp.append(t)
                for ntile in range(PANEL // 128):
                    ps = psum1.tile([128, W_DIM], f32, tag="ps")
                    for k in range(4):
                        nc.tensor.matmul(
                            ps[:],
                            lhsT=zp[k][:, ntile * 128 : (ntile + 1) * 128],
                            rhs=w_tiles[k][:],
                            start=(k == 0),
                            stop=(k == 3),
                        )
                    st = zwpool.tile([128, W_DIM], f16, tag="zwst")
                    nc.scalar.activation(
                        st[:], ps[:], mybir.ActivationFunctionType.Copy
                    )
                    node0 = p * PANEL + ntile * 128
                    nc.sync.dma_start(zw[node0 : node0 + 128, :], st[:])
            for ch in range(NCHUNK):
                icol = slice(ch * (CHUNK // 16), (ch + 1) * (CHUNK // 16))
                ct = cpool.tile([128, CHUNK // 128, W_DIM], f16, tag="ct")
                nc.gpsimd.dma_gather(
                    ct[:], ztbl[:], cidx_sb[:, icol], CHUNK, CHUNK, W_DIM
                )
                rt = rpool.tile([128, CHUNK // 128, W_DIM], f16, tag="rt")
                nc.gpsimd.dma_gather(
                    rt[:], zw[:], ridx_sb[:, icol], CHUNK, CHUNK, W_DIM
                )
                scr = prodpool.tile([128, CHUNK // 128, W_DIM], f16, tag="scr")
                nc.vector.tensor_mul(scr[:], rt[:], ct[:])
                nc.vector.tensor_reduce(
                    scores[:, ch * 8 : (ch + 1) * 8],
                    scr[:],
                    mybir.AxisListType.X,
                    mybir.AluOpType.add,
                )
            sig = mpool.tile([128, NBLK], f32, tag="sig")
            nc.scalar.activation(
                sig[:], scores[:], mybir.ActivationFunctionType.Sigmoid
            )
            nc.sync.dma_start(out[:], sig[:])
    nc.compile()
    return nc


def _get_nc(paired=True):
    key = "nc_paired" if paired else "nc_plain"
    if key not in _cache:
        _cache[key] = _build_paired() if paired else _build_plain()
    return _cache[key]


def _wrap_idx(idx):
    """int16 indices -> [128, n/16] layout: index i at [i%16, i//16],
    replicated across the 8 GPSIMD core groups (16 partitions each)."""
    blk = idx.reshape(-1, 16).T.astype(np.int16)  # [16, n/16]
    return np.ascontiguousarray(np.tile(blk, (8, 1)))  # [128, n/16]


def _pair_core(r_s, c_s):
    """Greedy chain pairing of row-sorted edges (gap 0/1 -> one 2KB desc).

    Returns (ridx_list[P_FIX+S_FIX], cidx_slots[EC], slot_edge[EC]) or
    None if the input pairs too poorly for the static layout."""
    n = len(r_s)
    pairs = []  # (ia, ib, delta)
    singles = []
    i = 0
    while i < n:
        if i + 1 < n and r_s[i + 1] - r_s[i] <= 1:
            pairs.append((i, i + 1, int(r_s[i + 1] - r_s[i])))
            i += 2
        else:
            singles.append(i)
            i += 1
    if len(pairs) < P_FIX:
        return None
    # demote the LOWEST-row pairs so demoted edges sit at the bottom of
    # the (row-sorted) singles range, keeping the first singles chunk's
    # zw2 prefix bound small
    n_extra = len(pairs) - P_FIX
    for ia, ib, _ in pairs[:n_extra]:
        singles.append(ia)
        singles.append(ib)
    pairs = pairs[n_extra:]

    bounds = _chunk_bounds()
    ridx_list = np.empty(P_FIX + S_FIX, dtype=np.int64)
    cidx_slots = np.empty(EC, dtype=np.int64)
    slot_edge = np.empty(EC, dtype=np.int64)
    for k in range(NPC):
        hi = 0
        for j in range(512):
            ia, ib, delta = pairs[k * 512 + j]
            s0 = k * CHUNK + 2 * (j // 128) * 128 + (j % 128)
            s1 = s0 + 128
            slot_edge[s0] = ia
            slot_edge[s1] = ib
            ridx_list[k * 512 + j] = 2 * int(r_s[ia]) + delta
            cidx_slots[s0] = c_s[ia]
            cidx_slots[s1] = c_s[ib]
            hi = max(hi, 2 * int(r_s[ia]) + delta)
        # view holds 2*bound-1 rows; idx must be <= 2*bound-2
        if hi > 2 * bounds[k] - 2:
            return None  # prefix bound violated; fall back
    singles.sort(key=lambda ie: int(r_s[ie]))
    if int(r_s[singles[511]]) >= SB0_NODES:
        return None  # singles bound violated; fall back
    base = NPC * CHUNK
    for t, ie in enumerate(singles):
        slot_edge[base + t] = ie
        ridx_list[P_FIX + t] = 2 * int(r_s[ie])
        cidx_slots[base + t] = c_s[ie]
    return ridx_list, cidx_slots, slot_edge


def _host_inputs(z, batch_edges, W):
    z = np.asarray(z, dtype=np.float32)
    W = np.asarray(W, dtype=np.float32)
    be = np.asarray(batch_edges)

    z_pad = np.zeros((N_NODES_PAD, W_DIM), dtype=np.float32)
    z_pad[:N_NODES] = z
    zt_np = np.ascontiguousarray(z_pad.T).astype(np.float16)
    ztbl_np = z_pad.astype(np.float16)
    w_np = W.astype(np.float16)

    rows = be[0].astype(np.int64)
    cols = be[1].astype(np.int64)

    orders = []
    pair_res = []
    paired = True
    for c in range(N_CORES):
        sl = slice(c * EC, (c + 1) * EC)
        order = np.argsort(rows[sl], kind="stable")
        orders.append(order)
        if paired:
            res = _pair_core(rows[sl][order], cols[sl][order])
            if res is None:
                paired = False
            else:
                pair_res.append(res)

    in_maps = []
    unsorts = []
    for c in range(N_CORES):
        sl = slice(c * EC, (c + 1) * EC)
        order = orders[c]
        if paired:
            ridx_list, cidx_slots, slot_edge = pair_res[c]
            unsorts.append(order[slot_edge])  # slot s -> original edge idx
        else:
            ridx_list = rows[sl][order]
            cidx_slots = cols[sl][order]
            unsorts.append(order)
        in_maps.append(
            {
                "zt": zt_np,
                "ztbl": ztbl_np,
                "w": w_np,
                "ridx": _wrap_idx(np.asarray(ridx_list)),
                "cidx": _wrap_idx(np.asarray(cidx_slots)),
            }
        )
    return in_maps, unsorts, paired


def kernel(z, batch_edges, W, _profile=False):
    from concourse.bass_utils import run_bass_kernel_spmd

    in_maps, unsorts, paired = _host_inputs(z, batch_edges, W)
    nc = _get_nc(paired=paired)
    kwargs = {}
    if _profile:
        kwargs = {"trace": True}
    res = run_bass_kernel_spmd(nc, in_maps, core_ids=list(range(N_CORES)), **kwargs)
    _cache["last_res"] = res

    outs = []
    for c in range(N_CORES):
        s = np.asarray(res.results[c]["scores"])  # [128, NBLK], slot s=(col*128+p)
        flat = np.ascontiguousarray(s.T).reshape(-1)
        unsorted = np.empty_like(flat)
        unsorted[unsorts[c]] = flat
        outs.append(unsorted)
    return np.concatenate(outs)


# revision 38
# speedup vs baseline: 1.1731x; 1.1731x over previous
"""Trainium2 Bass kernel for BilinearDecoder (v5).

score = sigmoid( einsum('ed,ed->e', z[edges[0]] @ W, z[edges[1]]) )

The kernel is bound by SWDGE descriptor generation on GPSIMD (~8ns per
gather descriptor, engine-serial; element SIZE is free).  v5 cuts row
descriptors ~2x by fetching edge PAIRS with one 2KB descriptor:

  Host sorts each core's edges by row; adjacent sorted edges have row
  gap 0 or 1 ~95% of the time.  A doubled table zw2 (flat 1KB rows:
  zw2[2i]=zW[i], zw2[2i+1]=zW[i]) serves both pair types with one
  overlapped-stride gather (elem 2KB, stride 1KB):
     idx 2r   -> [zW_r, zW_r ]   (equal-row pair)
     idx 2r+1 -> [zW_r, zW_r+1]  (consecutive-row pair)
  Pairs land as two 1KB halves in one partition; the col gather's index
  list is slot-permuted so each edge's z[col] row lands at the matching
  slot.  15 chunks x 512 pair-descs + 1 chunk x 1024 single-descs
  (P_FIX=7680 pairs, statically shaped; host falls back to the plain
  variant if an input pairs poorly) + 16 x 1024 col descs
  = 25088 descriptors vs 32768 plain.

  Phase 1 computes zW = z @ W in fp16 and writes each 128-node block
  twice (even/odd strided) into zw2; row-gather chunk k only reads a
  zw2 prefix (host-verified static bounds), so row gathers overlap the
  matmul via Tile's range-granular DRAM deps.  Per-edge dot: DVE f16
  mul + DVE tensor_reduce (keeps ACT off the critical path), ACT
  sigmoid at the end.
"""

import sys

if "/opt/trn_rl_repo" not in sys.path:
    sys.path.insert(0, "/opt/trn_rl_repo")

import numpy as np

N_NODES = 10000
N_NODES_PAD = 10240  # pad to multiple of 128
W_DIM = 512
N_EDGES = 131072
N_CORES = 8
EC = N_EDGES // N_CORES  # 16384 edges per core
CHUNK = 1024  # edge slots per chunk
NCHUNK = EC // CHUNK  # 16
NBLK = EC // 128  # 128 score columns per core
P_FIX = 7680  # static pair count: 15 chunks x 512 pairs
NPC = P_FIX // 512  # 15 pair chunks
S_FIX = EC - 2 * P_FIX  # 1024 single edges, 1 chunk
SB0_NODES = 7680  # zw2 prefix bound for the first 512 singles

_cache = {}


def _chunk_bounds():
    """Static per-pair-chunk zw prefix bounds (in nodes, mult of 128).

    Pair chunk k covers sorted edges up to ~(k+1)*CHUNK*(EC/(2*P_FIX)),
    i.e. rows below roughly that quantile.  +768 margin is >10 sigma of
    the order-statistic fluctuation; the host verifies per input and
    falls back to the plain variant."""
    bs = []
    for k in range(NPC):
        frac = (k + 1) * 2 * 512 / EC * (EC / (2 * P_FIX))  # = (k+1)/15
        b = int(np.ceil((N_NODES_PAD * frac + 768) / 128.0) * 128)
        bs.append(min(N_NODES_PAD, b))
    return bs


def _build_paired():
    import concourse.bacc as bacc
    import concourse.tile as tile
    from concourse import mybir
    import bass_rust

    f32 = mybir.dt.float32
    f16 = mybir.dt.float16
    i16 = mybir.dt.int16

    nc = bacc.Bacc(
        "TRN2",
        target_bir_lowering=False,
        debug=False,
        num_devices=N_CORES,
        dynamic_dma_scratch_size=16384,
    )
    zt = nc.dram_tensor("zt", [W_DIM, N_NODES_PAD], f16, kind="ExternalInput")
    ztbl = nc.dram_tensor("ztbl", [N_NODES_PAD, W_DIM], f16, kind="ExternalInput")
    w = nc.dram_tensor("w", [W_DIM, W_DIM], f16, kind="ExternalInput")
    # ridx: NPC*512 pair idxs then S_FIX single idxs (into zw2 flat rows)
    ridx = nc.dram_tensor(
        "ridx", [128, (P_FIX + S_FIX) // 16], i16, kind="ExternalInput"
    )
    cidx = nc.dram_tensor("cidx", [128, EC // 16], i16, kind="ExternalInput")
    zw2 = nc.dram_tensor("zw2", [2 * N_NODES_PAD, W_DIM], f16, kind="Internal")
    out = nc.dram_tensor("scores", [128, NBLK], f32, kind="ExternalOutput")

    def zw2_pair_view(bound_nodes):
        """Overlapped view: rows of 1024 f16 at 512-elem stride -> 2KB
        fetch at 1KB granularity.  Last row must not run off the buffer,
        so the view holds 2*bound-1 rows (idx <= 2*bound-2)."""
        nrows = 2 * bound_nodes - 1
        ap = zw2[:].copy()
        ap.ap = bass_rust.VecI64Pair([[W_DIM, nrows], [1, 2 * W_DIM]])
        return ap

    with tile.TileContext(nc) as tc:
        with (
            tc.tile_pool(name="wpool", bufs=1) as wpool,
            tc.tile_pool(name="zpanel", bufs=2) as zpool,
            tc.tile_pool(name="zwstage", bufs=4) as zwpool,
            tc.tile_pool(name="idx", bufs=1) as idxpool,
            tc.tile_pool(name="rgath", bufs=4) as rpool,
            tc.tile_pool(name="cgath", bufs=6) as cpool,
            tc.tile_pool(name="prod", bufs=4) as prodpool,
            tc.tile_pool(name="misc", bufs=1) as mpool,
            tc.tile_pool(name="psum1", bufs=4, space="PSUM") as psum1,
        ):
            w_tiles = []
            for k in range(4):
                wt = wpool.tile([128, W_DIM], f16, tag=f"w{k}")
                nc.sync.dma_start(wt[:], w[k * 128 : (k + 1) * 128, :])
                w_tiles.append(wt)

            ridx_sb = idxpool.tile([128, (P_FIX + S_FIX) // 16], i16, tag="ridx")
            nc.sync.dma_start(ridx_sb[:], ridx[:])
            cidx_sb = idxpool.tile([128, EC // 16], i16, tag="cidx")
            nc.sync.dma_start(cidx_sb[:], cidx[:])

            scores = mpool.tile([128, NBLK], f32, tag="scores")

            # ---- Phase 1: zW = z @ W (fp16), doubled into zw2 ----
            PANEL = 512
            for p in range(N_NODES_PAD // PANEL):
                zp = []
                for k in range(4):
                    t = zpool.tile([128, PANEL], f16, tag=f"zp{k}")
                    nc.sync.dma_start(
                        t[:], zt[k * 128 : (k + 1) * 128, p * PANEL : (p + 1) * PANEL]
                    )
                    zp.append(t)
                for ntile in range(PANEL // 128):
                    ps = psum1.tile([128, W_DIM], f32, tag="ps")
                    for k in range(4):
                        nc.tensor.matmul(
                            ps[:],
                            lhsT=zp[k][:, ntile * 128 : (ntile + 1) * 128],
                            rhs=w_tiles[k][:],
                            start=(k == 0),
                            stop=(k == 3),
                        )
                    st2 = zwpool.tile([128, 2 * W_DIM], f16, tag="zwst")
                    # ACT casts psum->f16 twice into [zw_i | zw_i] halves;
                    # partition p's 2KB then lands as zw2 rows 2i, 2i+1 in
                    # one contiguous DMA (keeps the Sync queue short).
                    nc.scalar.activation(
                        st2[:, 0:W_DIM], ps[:], mybir.ActivationFunctionType.Copy
                    )
                    nc.scalar.activation(
                        st2[:, W_DIM:], ps[:], mybir.ActivationFunctionType.Copy
                    )
                    node0 = p * PANEL + ntile * 128
                    nc.sync.dma_start(zw2[2 * node0 : 2 * node0 + 256, :], st2[:])

            # ---- Phase 2: paired/single row gathers + col gathers ----
            bounds = _chunk_bounds()
            for ch in range(NCHUNK):
                ct = cpool.tile([128, CHUNK // 128, W_DIM], f16, tag="ct")
                nc.gpsimd.dma_gather(
                    ct[:],
                    ztbl[:],
                    cidx_sb[:, ch * (CHUNK // 16) : (ch + 1) * (CHUNK // 16)],
                    CHUNK,
                    CHUNK,
                    W_DIM,
                )
                scr = prodpool.tile([128, CHUNK // 128, W_DIM], f16, tag="scr")
                if ch < NPC:
                    rt = rpool.tile([128, 4, 2 * W_DIM], f16, tag="rtp")
                    nc.gpsimd.dma_gather(
                        rt[:],
                        zw2_pair_view(bounds[ch]),
                        ridx_sb[:, ch * 32 : (ch + 1) * 32],
                        512,
                        512,
                        2 * W_DIM,
                        elem_step=W_DIM,
                    )
                    nc.vector.tensor_mul(
                        scr[:, 0:8:2, :], rt[:, :, 0:W_DIM], ct[:, 0:8:2, :]
                    )
                    nc.vector.tensor_mul(
                        scr[:, 1:8:2, :], rt[:, :, W_DIM:], ct[:, 1:8:2, :]
                    )
                else:
                    # two 512-desc single gathers; the first only needs a
                    # zw2 prefix (singles are row-sorted), so just the
                    # second waits for the full table.
                    rt1 = rpool.tile([128, 4, W_DIM], f16, tag="rts0")
                    nc.gpsimd.dma_gather(
                        rt1[:],
                        zw2[: 2 * SB0_NODES, :],
                        ridx_sb[:, NPC * 32 : NPC * 32 + 32],
                        512,
                        512,
                        W_DIM,
                    )
                    rt2 = rpool.tile([128, 4, W_DIM], f16, tag="rts1")
                    nc.gpsimd.dma_gather(
                        rt2[:],
                        zw2[:],
                        ridx_sb[:, NPC * 32 + 32 : NPC * 32 + 64],
                        512,
                        512,
                        W_DIM,
                    )
                    nc.vector.tensor_mul(scr[:, 0:4, :], rt1[:], ct[:, 0:4, :])
                    nc.vector.tensor_mul(scr[:, 4:8, :], rt2[:], ct[:, 4:8, :])
                nc.vector.tensor_reduce(
                    scores[:, ch * 8 : (ch + 1) * 8],
                    scr[:],
                    mybir.AxisListType.X,
                    mybir.AluOpType.add,
                )

            sig = mpool.tile([128, NBLK], f32, tag="sig")
            nc.scalar.activation(
                sig[:], scores[:], mybir.ActivationFunctionType.Sigmoid
            )
            nc.sync.dma_start(out[:], sig[:])

    nc.compile()
    return nc


def _build_plain():
    """Fallback for inputs that pair poorly: 32x1024 plain gathers
    (v1-style data path, fp16 phase 1, full-range row gathers)."""
    import concourse.bacc as bacc
    import concourse.tile as tile
    from concourse import mybir

    f32 = mybir.dt.float32
    f16 = mybir.dt.float16
    i16 = mybir.dt.int16

    nc = bacc.Bacc(
        "TRN2",
        target_bir_lowering=False,
        debug=False,
        num_devices=N_CORES,
        dynamic_dma_scratch_size=16384,
    )
    zt = nc.dram_tensor("zt", [W_DIM, N_NODES_PAD], f16, kind="ExternalInput")
    ztbl = nc.dram_tensor("ztbl", [N_NODES_PAD, W_DIM], f16, kind="ExternalInput")
    w = nc.dram_tensor("w", [W_DIM, W_DIM], f16, kind="ExternalInput")
    ridx = nc.dram_tensor("ridx", [128, EC // 16], i16, kind="ExternalInput")
    cidx = nc.dram_tensor("cidx", [128, EC // 16], i16, kind="ExternalInput")
    zw = nc.dram_tensor("zw", [N_NODES_PAD, W_DIM], f16, kind="Internal")
    out = nc.dram_tensor("scores", [128, NBLK], f32, kind="ExternalOutput")

    with tile.TileContext(nc) as tc:
        with (
            tc.tile_pool(name="wpool", bufs=1) as wpool,
            tc.tile_pool(name="zpanel", bufs=2) as zpool,
            tc.tile_pool(name="zwstage", bufs=4) as zwpool,
            tc.tile_pool(name="idx", bufs=1) as idxpool,
            tc.tile_pool(name="rgath", bufs=2) as rpool,
            tc.tile_pool(name="cgath", bufs=2) as cpool,
            tc.tile_pool(name="prod", bufs=2) as prodpool,
            tc.tile_pool(name="misc", bufs=1) as mpool,
            tc.tile_pool(name="psum1", bufs=4, space="PSUM") as psum1,
        ):
            w_tiles = []
            for k in range(4):
                wt = wpool.tile([128, W_DIM], f16, tag=f"w{k}")
                nc.sync.dma_start(wt[:], w[k * 128 : (k + 1) * 128, :])
                w_tiles.append(wt)
            ridx_sb = idxpool.tile([128, EC // 16], i16, tag="ridx")
            nc.sync.dma_start(ridx_sb[:], ridx[:])
            cidx_sb = idxpool.tile([128, EC // 16], i16, tag="cidx")
            nc.sync.dma_start(cidx_sb[:], cidx[:])
            scores = mpool.tile([128, NBLK], f32, tag="scores")
            PANEL = 512
            for p in range(N_NODES_PAD // PANEL):
                zp = []
                for k in range(4):
                    t = zpool.tile([128, PANEL], f16, tag=f"zp{k}")
                    nc.sync.dma_start(
                        t[:], zt[k * 128 : (k + 1) * 128, p * PANEL : (p + 1) * PANEL]
                    )
                    zp.append(t)
                for ntile in range(PANEL // 128):
                    ps = psum1.tile([128, W_DIM], f32, tag="ps")
                    for k in range(4):
                        nc.tensor.matmul(
                            ps[:],
                            lhsT=zp[k][:, ntile * 128 : (ntile + 1) * 128],
                            rhs=w_tiles[k][:],
                            start=(k == 0),
                            stop=(k == 3),
                        )
                    st = zwpool.tile([128, W_DIM], f16, tag="zwst")
                    nc.scalar.activation(
                        st[:], ps[:], mybir.ActivationFunctionType.Copy
                    )
                    node0 = p * PANEL + ntile * 128
                    nc.sync.dma_start(zw[node0 : node0 + 128, :], st[:])
            for ch in range(NCHUNK):
                icol = slice(ch * (CHUNK // 16), (ch + 1) * (CHUNK // 16))
                ct = cpool.tile([128, CHUNK // 128, W_DIM], f16, tag="ct")
                nc.gpsimd.dma_gather(
                    ct[:], ztbl[:], cidx_sb[:, icol], CHUNK, CHUNK, W_DIM
                )
                rt = rpool.tile([128, CHUNK // 128, W_DIM], f16, tag="rt")
                nc.gpsimd.dma_gather(
                    rt[:], zw[:], ridx_sb[:, icol], CHUNK, CHUNK, W_DIM
                )
                scr = prodpool.tile([128, CHUNK // 128, W_DIM], f16, tag="scr")
                nc.vector.tensor_mul(scr[:], rt[:], ct[:])
                nc.vector.tensor_reduce(
                    scores[:, ch * 8 : (ch + 1) * 8],
                    scr[:],
                    mybir.AxisListType.X,
                    mybir.AluOpType.add,
                )
            sig = mpool.tile([128, NBLK], f32, tag="sig")
            nc.scalar.activation(
                sig[:], scores[:], mybir.ActivationFunctionType.Sigmoid
            )
            nc.sync.dma_start(out[:], sig[:])
    nc.compile()
    return nc


def _get_nc(paired=True):
    key = "nc_paired" if paired else "nc_plain"
    if key not in _cache:
        _cache[key] = _build_paired() if paired else _build_plain()
    return _cache[key]


def _wrap_idx(idx):
    """int16 indices -> [128, n/16] layout: index i at [i%16, i//16],
    replicated across the 8 GPSIMD core groups (16 partitions each)."""
    blk = idx.reshape(-1, 16).T.astype(np.int16)  # [16, n/16]
    return np.ascontiguousarray(np.tile(blk, (8, 1)))  # [128, n/16]


def _pair_core(r_s, c_s):
    """Greedy chain pairing of row-sorted edges (gap 0/1 -> one 2KB desc).

    Returns (ridx_list[P_FIX+S_FIX], cidx_slots[EC], slot_edge[EC]) or
    None if the input pairs too poorly for the static layout."""
    n = len(r_s)
    pairs = []  # (ia, ib, delta)
    singles = []
    i = 0
    while i < n:
        if i + 1 < n and r_s[i + 1] - r_s[i] <= 1:
            pairs.append((i, i + 1, int(r_s[i + 1] - r_s[i])))
            i += 2
        else:
            singles.append(i)
            i += 1
    if len(pairs) < P_FIX:
        return None
    # demote the LOWEST-row pairs so demoted edges sit at the bottom of
    # the (row-sorted) singles range, keeping the first singles chunk's
    # zw2 prefix bound small
    n_extra = len(pairs) - P_FIX
    for ia, ib, _ in pairs[:n_extra]:
        singles.append(ia)
        singles.append(ib)
    pairs = pairs[n_extra:]

    bounds = _chunk_bounds()
    ridx_list = np.empty(P_FIX + S_FIX, dtype=np.int64)
    cidx_slots = np.empty(EC, dtype=np.int64)
    slot_edge = np.empty(EC, dtype=np.int64)
    for k in range(NPC):
        hi = 0
        for j in range(512):
            ia, ib, delta = pairs[k * 512 + j]
            s0 = k * CHUNK + 2 * (j // 128) * 128 + (j % 128)
            s1 = s0 + 128
            slot_edge[s0] = ia
            slot_edge[s1] = ib
            ridx_list[k * 512 + j] = 2 * int(r_s[ia]) + delta
            cidx_slots[s0] = c_s[ia]
            cidx_slots[s1] = c_s[ib]
            hi = max(hi, 2 * int(r_s[ia]) + delta)
        # view holds 2*bound-1 rows; idx must be <= 2*bound-2
        if hi > 2 * bounds[k] - 2:
            return None  # prefix bound violated; fall back
    singles.sort(key=lambda ie: int(r_s[ie]))
    if int(r_s[singles[511]]) >= SB0_NODES:
        return None  # singles bound violated; fall back
    base = NPC * CHUNK
    for t, ie in enumerate(singles):
        slot_edge[base + t] = ie
        ridx_list[P_FIX + t] = 2 * int(r_s[ie])
        cidx_slots[base + t] = c_s[ie]
    return ridx_list, cidx_slots, slot_edge


def _host_inputs(z, batch_edges, W):
    z = np.asarray(z, dtype=np.float32)
    W = np.asarray(W, dtype=np.float32)
    be = np.asarray(batch_edges)

    z_pad = np.zeros((N_NODES_PAD, W_DIM), dtype=np.float32)
    z_pad[:N_NODES] = z
    zt_np = np.ascontiguousarray(z_pad.T).astype(np.float16)
    ztbl_np = z_pad.astype(np.float16)
    w_np = W.astype(np.float16)

    rows = be[0].astype(np.int64)
    cols = be[1].astype(np.int64)

    orders = []
    pair_res = []
    paired = True
    for c in range(N_CORES):
        sl = slice(c * EC, (c + 1) * EC)
        order = np.argsort(rows[sl], kind="stable")
        orders.append(order)
        if paired:
            res = _pair_core(rows[sl][order], cols[sl][order])
            if res is None:
                paired = False
            else:
                pair_res.append(res)

    in_maps = []
    unsorts = []
    for c in range(N_CORES):
        sl = slice(c * EC, (c + 1) * EC)
        order = orders[c]
        if paired:
            ridx_list, cidx_slots, slot_edge = pair_res[c]
            unsorts.append(order[slot_edge])  # slot s -> original edge idx
        else:
            ridx_list = rows[sl][order]
            cidx_slots = cols[sl][order]
            unsorts.append(order)
        in_maps.append(
            {
                "zt": zt_np,
                "ztbl": ztbl_np,
                "w": w_np,
                "ridx": _wrap_idx(np.asarray(ridx_list)),
                "cidx": _wrap_idx(np.asarray(cidx_slots)),
            }
        )
    return in_maps, unsorts, paired


def kernel(z, batch_edges, W, _profile=False):
    from concourse.bass_utils import run_bass_kernel_spmd

    in_maps, unsorts, paired = _host_inputs(z, batch_edges, W)
    nc = _get_nc(paired=paired)
    kwargs = {}
    if _profile:
        kwargs = {"trace": True}
    res = run_bass_kernel_spmd(nc, in_maps, core_ids=list(range(N_CORES)), **kwargs)
    _cache["last_res"] = res

    outs = []
    for c in range(N_CORES):
        s = np.asarray(res.results[c]["scores"])  # [128, NBLK], slot s=(col*128+p)
        flat = np.ascontiguousarray(s.T).reshape(-1)
        unsorted = np.empty_like(flat)
        unsorted[unsorts[c]] = flat
        outs.append(unsorted)
    return np.concatenate(outs)
